# revision 2
# baseline (speedup 1.0000x reference)
"""Single-head attention  softmax(Q K^T / sqrt(64)) V  on 8 TRN2 NeuronCores.

Shapes: Q, K, V = [8192, 64] f32; output [8192, 64] f32.

Sharding: Q rows split 8 ways (1024 rows/core); K, V replicated.

Per-core algorithm (transpose-free layout):
  - scoresT[k, q] tiles ([128 k x 512 q]) via matmul with lhsT = K^T tile
    (d=64 contraction on partitions), rhs = Q^T block.  fp32r operands
    (full-rate PE, ~1.6e-4 rel err vs 4x-slower exact fp32).
  - exp on ScalarE straight out of PSUM, with the 1/sqrt(64) scale fused
    into the activation's free affine.  No max-subtraction: scores are
    ~N(0,1) so exp is safely in fp32 range, matching softmax exactly.
  - second matmul accumulates [V | 1] (ones-augmented, k-tiled) against the
    exp tiles into a [65, 512] PSUM accumulator: rows 0..63 = unnormalized
    out^T, row 64 = softmax denominator.  One accumulation over all 64
    k-tiles per q-block.
  - device returns OT [65, 1024] per core; host divides by row 64 and
    transposes.  Host also pre-transposes/packs Q^T, K^T, [V|1] so the
    device does zero transposes.

K^T is packed [128, 4096] (two 64-row halves side by side) and Q^T is
duplicated into both partition halves so k-tiles >= 32 contract on array
rows 64..127 (full DMA width, base-partition rule satisfied).
"""

import numpy as np

N = 8192
D = 64
N_CORES = 8
QL = N // N_CORES          # 1024 q rows per core
KT_TILES = N // 128        # 64 k-tiles of 128
QB = 512                   # q block (one PSUM bank of f32)
KB = 3                     # k-tiles per exp batch (3 PSUM banks)

_CACHE = {}


def build_body(nc, tile, mybir, QT, KT, VA, OT, iters=1):
    """Emit the per-core program. QT [128, QL] (Q^T/dup), KT [128, N/2]
    (packed K^T), VA [128, 64*65] (k-tiled [V|1]), OT [65, QL]."""
    from contextlib import ExitStack

    F32 = mybir.dt.float32
    F32R = mybir.dt.float32r
    EXP = mybir.ActivationFunctionType.Exp
    n_qh = QL // QB

    with tile.TileContext(nc) as tc, ExitStack() as ctx:
        sing = ctx.enter_context(tc.tile_pool(name="sing", bufs=1))
        scpool = ctx.enter_context(tc.tile_pool(name="sc", bufs=2, space="PSUM"))
        acpool = ctx.enter_context(tc.tile_pool(name="ac", bufs=2, space="PSUM"))
        expool = ctx.enter_context(tc.tile_pool(name="ex", bufs=3))
        outpool = ctx.enter_context(tc.tile_pool(name="ot", bufs=2))

        qt = sing.tile([128, QL], F32R, tag="qt")
        kt = sing.tile([128, N // 2], F32R, tag="kt")
        va = sing.tile([128, KT_TILES * (D + 1)], F32R, tag="va")

        nc.sync.dma_start(qt[:], QT[:])
        for i in range(8):
            w = (N // 2) // 8
            nc.sync.dma_start(kt[:, i * w:(i + 1) * w], KT[:, i * w:(i + 1) * w])
        for i in range(8):
            w = KT_TILES * (D + 1) // 8
            nc.sync.dma_start(va[:, i * w:(i + 1) * w], VA[:, i * w:(i + 1) * w])

        for _ in range(iters):
            ot_sb = outpool.tile([65, QL], F32, tag="ot")
            for qh in range(n_qh):
                o2 = acpool.tile([65, QB], F32, tag="o2")
                k = 0
                while k < KT_TILES:
                    bsz = min(KB, KT_TILES - k)
                    sc = scpool.tile([128, QB * KB], F32, tag="sc")
                    for j in range(bsz):
                        kk = k + j
                        half = 0 if kk < 32 else 64
                        col = (kk % 32) * 128
                        nc.tensor.matmul(
                            sc[:, j * QB:(j + 1) * QB],
                            kt[half:half + 64, col:col + 128],
                            qt[half:half + 64, qh * QB:(qh + 1) * QB],
                            start=True, stop=True,
                        )
                    ex = expool.tile([128, QB * KB], F32R, tag="ex")
                    nc.scalar.activation(
                        ex[:, :bsz * QB], sc[:, :bsz * QB], EXP, scale=0.125,
                    )
                    for j in range(bsz):
                        kk = k + j
                        nc.tensor.matmul(
                            o2[:],
                            va[:, kk * (D + 1):(kk + 1) * (D + 1)],
                            ex[:, j * QB:(j + 1) * QB],
                            start=(kk == 0), stop=(kk == KT_TILES - 1),
                        )
                    k += bsz
                nc.vector.tensor_copy(ot_sb[:, qh * QB:(qh + 1) * QB], o2[:])
            nc.sync.dma_start(OT[:], ot_sb[:])


def _build(iters=1):
    key = ("nc", iters)
    if key in _CACHE:
        return _CACHE[key]
    import concourse.tile as tile
    from concourse import bacc, mybir

    F32 = mybir.dt.float32
    F32R = mybir.dt.float32r
    nc = bacc.Bacc("TRN2", target_bir_lowering=False, debug=False,
                   num_devices=N_CORES)
    QT = nc.dram_tensor("QT", [128, QL], F32R, kind="ExternalInput").ap()
    KT = nc.dram_tensor("KT", [128, N // 2], F32R, kind="ExternalInput").ap()
    VA = nc.dram_tensor("VA", [128, KT_TILES * (D + 1)], F32R,
                        kind="ExternalInput").ap()
    OT = nc.dram_tensor("OT", [65, QL], F32, kind="ExternalOutput").ap()
    build_body(nc, tile, mybir, QT, KT, VA, OT, iters=iters)
    nc.compile()
    _CACHE[key] = nc
    return nc


def prep_inputs(Q, K, V):
    """Host-side shard/pack. Returns per-core input maps."""
    Q = np.ascontiguousarray(np.asarray(Q, dtype=np.float32))
    K = np.ascontiguousarray(np.asarray(K, dtype=np.float32))
    V = np.ascontiguousarray(np.asarray(V, dtype=np.float32))

    KTf = K.T                                   # [64, 8192]
    KTp = np.concatenate([KTf[:, :N // 2], KTf[:, N // 2:]], axis=0)  # [128, 4096]
    KTp = np.ascontiguousarray(KTp)

    Vaug = np.ones((N, D + 1), dtype=np.float32)
    Vaug[:, :D] = V
    # [128, kt*(D+1)] : partition p, tile t -> V row t*128+p
    VAp = np.ascontiguousarray(
        Vaug.reshape(KT_TILES, 128, D + 1).transpose(1, 0, 2).reshape(128, -1))

    in_maps = []
    for c in range(N_CORES):
        QTc = Q[c * QL:(c + 1) * QL].T          # [64, QL]
        QTd = np.ascontiguousarray(np.concatenate([QTc, QTc], axis=0))
        in_maps.append({"QT": QTd, "KT": KTp, "VA": VAp})
    return in_maps


def postprocess(results):
    """Divide by softmax denominator and transpose back, per core."""
    outs = []
    for c in range(N_CORES):
        OTc = results[c]["OT"]                  # [65, QL]
        outs.append((OTc[:D] / OTc[D:D + 1]).T)
    return np.ascontiguousarray(np.concatenate(outs, axis=0), dtype=np.float32)


def kernel(Q, K, V):
    from concourse.bass_utils import run_bass_kernel_spmd

    nc = _build(iters=1)
    in_maps = prep_inputs(Q, K, V)
    res = run_bass_kernel_spmd(nc, in_maps, list(range(N_CORES)))
    return postprocess(res.results)


# revision 38
# speedup vs baseline: 282.9483x; 282.9483x over previous
"""Single-head attention  softmax(Q K^T / sqrt(64)) V  on 8 TRN2 NeuronCores.

Shapes: Q, K, V = [8192, 64] f32; output [8192, 64] f32.

Sharding: Q rows split 8 ways (1024 rows/core); K, V replicated.

Per-core algorithm (transpose-free layout):
  - scoresT[k, q] tiles ([128 k x 512 q]) via matmul with lhsT = K^T tile
    (d=64 contraction on partitions), rhs = Q^T block.  fp16 operands:
    full-rate PE streaming (fp32 is 4x slower, fp32r measured 2x slower on
    HW despite the cost model's claim) at ~1e-3 overall rel err.
  - exp on ScalarE straight out of PSUM, with the 1/sqrt(64) scale fused
    into the activation's free affine.  No max-subtraction: scores are
    ~N(0,1) so exp is safely in fp32 range, matching softmax exactly.
  - second matmul accumulates [V | 1] (ones-augmented, k-tiled) against the
    exp tiles into a [65, 512] PSUM accumulator: rows 0..63 = unnormalized
    out^T, row 64 = softmax denominator.  One accumulation over all 64
    k-tiles per q-block.
  - device returns OT [65, 1024] per core; host divides by row 64 and
    transposes.  Host also pre-transposes/packs Q^T, K^T, [V|1] so the
    device does zero transposes.

K^T is packed [128, 4096] (two 64-row halves side by side) and Q^T is
duplicated into both partition halves so k-tiles >= 32 contract on array
rows 64..127 (full DMA width, base-partition rule satisfied).
"""

import numpy as np

N = 8192
D = 64
N_CORES = 8
QL = N // N_CORES          # 1024 q rows per core
KT_TILES = N // 128        # 64 k-tiles of 128
QB = 512                   # q block (one PSUM bank of f32)
KB = 3                     # k-tiles per exp batch (3 PSUM banks)

_CACHE = {}


def build_body(nc, tile, mybir, QT, KT, VA, OT, iters=1, loop_iters=1,
               mov_dt="float16", mode="full", kb=KB, sc_bufs=2, ac_bufs=2,
               ex_bufs=4):
    """Emit the per-core program. QT [128, QL] (Q^T/dup), KT [128, N/2]
    (packed K^T), VA [128, 64*65] (k-tiled [V|1]), OT [65, QL].

    iters: unrolled repetitions of the whole computation (timing runs).
    loop_iters: hardware For_i loop around those repetitions (timing runs).
    """
    from contextlib import ExitStack

    F32 = mybir.dt.float32
    F32R = mybir.dt.float32r
    MOV = getattr(mybir.dt, mov_dt)
    EXP = mybir.ActivationFunctionType.Exp
    n_qh = QL // QB

    with tile.TileContext(nc) as tc, ExitStack() as ctx:
        sing = ctx.enter_context(tc.tile_pool(name="sing", bufs=1))
        scpool = ctx.enter_context(
            tc.tile_pool(name="sc", bufs=sc_bufs, space="PSUM"))
        acpool = ctx.enter_context(
            tc.tile_pool(name="ac", bufs=ac_bufs, space="PSUM"))
        expool = ctx.enter_context(tc.tile_pool(name="ex", bufs=ex_bufs))
        outpool = ctx.enter_context(tc.tile_pool(name="ot", bufs=2))

        qt = sing.tile([128, QL], MOV, tag="qt")
        kt = sing.tile([128, N // 2], MOV, tag="kt")
        va = sing.tile([128, KT_TILES * (D + 1)], MOV, tag="va")

        nc.sync.dma_start(qt[:], QT[:])
        for i in range(8):
            w = (N // 2) // 8
            nc.sync.dma_start(kt[:, i * w:(i + 1) * w], KT[:, i * w:(i + 1) * w])
        for i in range(8):
            w = KT_TILES * (D + 1) // 8
            nc.sync.dma_start(va[:, i * w:(i + 1) * w], VA[:, i * w:(i + 1) * w])

        def emit_iter_kb4():
            # scores [128, 2048] x2 slots = 8 banks; mm2 batch partials are
            # written back into bank 0 of the spent scores slot, then DVE
            # accumulates them into an SBUF accumulator.
            ot_sb = outpool.tile([65, QL], F32, tag="ot")
            for qh in range(n_qh):
                acc = expool.tile([65, QB], F32, tag="acc")
                nc.vector.memset(acc[:], 0.0)
                for b in range(KT_TILES // 4):
                    k0 = b * 4
                    sc = scpool.tile([128, QB * 4], F32, tag="sc")
                    for j in range(4):
                        kk = k0 + j
                        half = 0 if kk < 32 else 64
                        col = (kk % 32) * 128
                        nc.tensor.matmul(
                            sc[:, j * QB:(j + 1) * QB],
                            kt[half:half + 64, col:col + 128],
                            qt[half:half + 64, qh * QB:(qh + 1) * QB],
                            start=True, stop=True,
                        )
                    ex = expool.tile([128, QB * 4], MOV, tag="ex")
                    nc.scalar.activation(ex[:], sc[:], EXP, scale=0.125)
                    part = sc[0:65, 0:QB]
                    for j in range(4):
                        kk = k0 + j
                        nc.tensor.matmul(
                            part,
                            va[:, kk * (D + 1):(kk + 1) * (D + 1)],
                            ex[:, j * QB:(j + 1) * QB],
                            start=(j == 0), stop=(j == 3),
                        )
                    nc.vector.tensor_add(acc[:], acc[:], part)
                nc.vector.tensor_copy(ot_sb[:, qh * QB:(qh + 1) * QB], acc[:])
            nc.sync.dma_start(OT[:], ot_sb[:])

        def emit_iter():
            if mode == "kb4":
                emit_iter_kb4()
                return
            ot_sb = outpool.tile([65, QL], F32, tag="ot")
            for qh in range(n_qh):
                o2 = acpool.tile([65, QB], F32, tag="o2")
                batches = []
                k = 0
                while k < KT_TILES:
                    batches.append((k, min(kb, KT_TILES - k)))
                    k += kb

                def emit_mm2(ex, k0, bsz):
                    for j in range(bsz):
                        kk = k0 + j
                        nc.tensor.matmul(
                            o2[:],
                            va[:, kk * (D + 1):(kk + 1) * (D + 1)],
                            ex[:, j * QB:(j + 1) * QB],
                            start=(kk == 0), stop=(kk == KT_TILES - 1),
                        )

                for k0, bsz in batches:
                    sc = scpool.tile([128, QB * kb], F32, tag="sc")
                    for j in range(bsz):
                        kk = k0 + j
                        half = 0 if kk < 32 else 64
                        col = (kk % 32) * 128
                        nc.tensor.matmul(
                            sc[:, j * QB:(j + 1) * QB],
                            kt[half:half + 64, col:col + 128],
                            qt[half:half + 64, qh * QB:(qh + 1) * QB],
                            start=True, stop=True,
                        )
                    if mode == "noexp":
                        continue
                    ex = expool.tile([128, QB * kb], MOV, tag="ex")
                    nc.scalar.activation(
                        ex[:, :bsz * QB], sc[:, :bsz * QB], EXP, scale=0.125,
                    )
                    if mode == "nomm2":
                        continue
                    emit_mm2(ex, k0, bsz)
                if mode == "full":
                    nc.vector.tensor_copy(ot_sb[:, qh * QB:(qh + 1) * QB], o2[:])
            if mode == "full":
                nc.sync.dma_start(OT[:], ot_sb[:])

        if loop_iters > 1:
            with tc.For_i(0, loop_iters, 1):
                for _ in range(iters):
                    emit_iter()
        else:
            for _ in range(iters):
                emit_iter()


def _build(iters=1, loop_iters=1, mov_dt="float16", num_devices=N_CORES,
           mode="full", kb=KB, sc_bufs=2, ac_bufs=2, ex_bufs=4):
    key = ("nc", iters, loop_iters, mov_dt, num_devices, mode,
           kb, sc_bufs, ac_bufs, ex_bufs)
    if key in _CACHE:
        return _CACHE[key]
    import concourse.tile as tile
    from concourse import bacc, mybir

    F32 = mybir.dt.float32
    F32R = mybir.dt.float32r
    MOV = getattr(mybir.dt, mov_dt)
    nc = bacc.Bacc("TRN2", target_bir_lowering=False, debug=False,
                   num_devices=num_devices)
    QT = nc.dram_tensor("QT", [128, QL], MOV, kind="ExternalInput").ap()
    KT = nc.dram_tensor("KT", [128, N // 2], MOV, kind="ExternalInput").ap()
    VA = nc.dram_tensor("VA", [128, KT_TILES * (D + 1)], MOV,
                        kind="ExternalInput").ap()
    OT = nc.dram_tensor("OT", [65, QL], F32, kind="ExternalOutput").ap()
    build_body(nc, tile, mybir, QT, KT, VA, OT, iters=iters,
               loop_iters=loop_iters, mov_dt=mov_dt, mode=mode,
               kb=kb, sc_bufs=sc_bufs, ac_bufs=ac_bufs, ex_bufs=ex_bufs)
    nc.compile()
    _CACHE[key] = nc
    return nc


def prep_inputs(Q, K, V, mov_np=np.float16):
    """Host-side shard/pack. Returns per-core input maps."""
    Q = np.ascontiguousarray(np.asarray(Q, dtype=np.float32))
    K = np.ascontiguousarray(np.asarray(K, dtype=np.float32))
    V = np.ascontiguousarray(np.asarray(V, dtype=np.float32))

    KTf = K.T                                   # [64, 8192]
    KTp = np.concatenate([KTf[:, :N // 2], KTf[:, N // 2:]], axis=0)  # [128, 4096]
    KTp = np.ascontiguousarray(KTp.astype(mov_np))

    Vaug = np.ones((N, D + 1), dtype=np.float32)
    Vaug[:, :D] = V
    # [128, kt*(D+1)] : partition p, tile t -> V row t*128+p
    VAp = np.ascontiguousarray(
        Vaug.reshape(KT_TILES, 128, D + 1).transpose(1, 0, 2)
            .reshape(128, -1).astype(mov_np))

    in_maps = []
    for c in range(N_CORES):
        QTc = Q[c * QL:(c + 1) * QL].T          # [64, QL]
        QTd = np.ascontiguousarray(
            np.concatenate([QTc, QTc], axis=0).astype(mov_np))
        in_maps.append({"QT": QTd, "KT": KTp, "VA": VAp})
    return in_maps


def postprocess(results):
    """Divide by softmax denominator and transpose back, per core."""
    outs = []
    for c in range(N_CORES):
        OTc = results[c]["OT"]                  # [65, QL]
        outs.append((OTc[:D] / OTc[D:D + 1]).T)
    return np.ascontiguousarray(np.concatenate(outs, axis=0), dtype=np.float32)


def kernel(Q, K, V):
    from concourse.bass_utils import run_bass_kernel_spmd

    nc = _build(iters=1)
    in_maps = prep_inputs(Q, K, V)
    res = run_bass_kernel_spmd(nc, in_maps, list(range(N_CORES)))
    return postprocess(res.results)


# revision 44
# speedup vs baseline: 345.4522x; 1.2209x over previous
"""Single-head attention  softmax(Q K^T / sqrt(64)) V  on 8 TRN2 NeuronCores.

Shapes: Q, K, V = [8192, 64] f32; output [8192, 64] f32.

Sharding: Q rows split 8 ways (1024 rows/core); K, V replicated.

Per-core algorithm (transpose-free layout):
  - scoresT[k, q] tiles ([128 k x 512 q]) via matmul with lhsT = K^T tile
    (d=64 contraction on partitions), rhs = Q^T block.  fp16 operands:
    full-rate PE streaming (fp32 is 4x slower, fp32r measured 2x slower on
    HW despite the cost model's claim) at ~1e-3 overall rel err.
  - exp on ScalarE straight out of PSUM, with the 1/sqrt(64) scale fused
    into the activation's free affine.  No max-subtraction: scores are
    ~N(0,1) so exp is safely in fp32 range, matching softmax exactly.
  - second matmul accumulates [V | 1] (ones-augmented, k-tiled) against the
    exp tiles into a [65, 512] PSUM accumulator: rows 0..63 = unnormalized
    out^T, row 64 = softmax denominator.  One accumulation over all 64
    k-tiles per q-block.
  - device returns OT [65, 1024] per core; host divides by row 64 and
    transposes.  Host also pre-transposes/packs Q^T, K^T, [V|1] so the
    device does zero transposes.

K^T is packed [128, 4096] (two 64-row halves side by side) and Q^T is
duplicated into both partition halves so k-tiles >= 32 contract on array
rows 64..127 (full DMA width, base-partition rule satisfied).

Default schedule ("ilv2"): the two 512-wide q-block chains are interleaved
batch-by-batch (independent work fills cross-engine sem-wait gaps), and
k-tiles are enumerated 0,32,1,33,... so consecutive mm1 stationaries target
opposite PE row halves, letting the PE's reorder window pull each LDWEIGHTS
ahead of the in-flight matmul (~12 us/core on HW vs sequential halves).
"""

import numpy as np

N = 8192
D = 64
N_CORES = 8
QL = N // N_CORES          # 1024 q rows per core
KT_TILES = N // 128        # 64 k-tiles of 128
QB = 512                   # q block (one PSUM bank of f32)
KB = 3                     # k-tiles per exp batch (3 PSUM banks)

_CACHE = {}


def build_body(nc, tile, mybir, QT, KT, VA, OT, iters=1, loop_iters=1,
               mov_dt="float16", mode="ilv2", kb=KB, sc_bufs=2, ac_bufs=2,
               ex_bufs=4):
    """Emit the per-core program. QT [128, QL] (Q^T/dup), KT [128, N/2]
    (packed K^T), VA [128, 64*65] (k-tiled [V|1]), OT [65, QL].

    iters: unrolled repetitions of the whole computation (timing runs).
    loop_iters: hardware For_i loop around those repetitions (timing runs).
    """
    from contextlib import ExitStack

    F32 = mybir.dt.float32
    F32R = mybir.dt.float32r
    MOV = getattr(mybir.dt, mov_dt)
    EXP = mybir.ActivationFunctionType.Exp
    n_qh = QL // QB

    with tile.TileContext(nc) as tc, ExitStack() as ctx:
        sing = ctx.enter_context(tc.tile_pool(name="sing", bufs=1))
        scpool = ctx.enter_context(
            tc.tile_pool(name="sc", bufs=sc_bufs, space="PSUM"))
        acpool = ctx.enter_context(
            tc.tile_pool(name="ac", bufs=ac_bufs, space="PSUM"))
        expool = ctx.enter_context(tc.tile_pool(name="ex", bufs=ex_bufs))
        outpool = ctx.enter_context(tc.tile_pool(name="ot", bufs=2))

        qt = sing.tile([128, QL], MOV, tag="qt")
        kt = sing.tile([128, N // 2], MOV, tag="kt")
        va = sing.tile([128, KT_TILES * (D + 1)], MOV, tag="va")

        nc.sync.dma_start(qt[:], QT[:])
        for i in range(8):
            w = (N // 2) // 8
            nc.sync.dma_start(kt[:, i * w:(i + 1) * w], KT[:, i * w:(i + 1) * w])
        for i in range(8):
            w = KT_TILES * (D + 1) // 8
            nc.sync.dma_start(va[:, i * w:(i + 1) * w], VA[:, i * w:(i + 1) * w])

        def emit_iter_kb4():
            # scores [128, 2048] x2 slots = 8 banks; mm2 batch partials are
            # written back into bank 0 of the spent scores slot, then DVE
            # accumulates them into an SBUF accumulator.
            ot_sb = outpool.tile([65, QL], F32, tag="ot")
            for qh in range(n_qh):
                acc = expool.tile([65, QB], F32, tag="acc")
                nc.vector.memset(acc[:], 0.0)
                for b in range(KT_TILES // 4):
                    k0 = b * 4
                    sc = scpool.tile([128, QB * 4], F32, tag="sc")
                    for j in range(4):
                        kk = k0 + j
                        half = 0 if kk < 32 else 64
                        col = (kk % 32) * 128
                        nc.tensor.matmul(
                            sc[:, j * QB:(j + 1) * QB],
                            kt[half:half + 64, col:col + 128],
                            qt[half:half + 64, qh * QB:(qh + 1) * QB],
                            start=True, stop=True,
                        )
                    ex = expool.tile([128, QB * 4], MOV, tag="ex")
                    nc.scalar.activation(ex[:], sc[:], EXP, scale=0.125)
                    part = sc[0:65, 0:QB]
                    for j in range(4):
                        kk = k0 + j
                        nc.tensor.matmul(
                            part,
                            va[:, kk * (D + 1):(kk + 1) * (D + 1)],
                            ex[:, j * QB:(j + 1) * QB],
                            start=(j == 0), stop=(j == 3),
                        )
                    nc.vector.tensor_add(acc[:], acc[:], part)
                nc.vector.tensor_copy(ot_sb[:, qh * QB:(qh + 1) * QB], acc[:])
            nc.sync.dma_start(OT[:], ot_sb[:])

        def emit_iter_ilv(alt_halves):
            # Interleave the two independent q-block chains batch-by-batch so
            # each engine has the other chain's work to fill sem-wait gaps.
            # alt_halves: enumerate k-tiles as 0,32,1,33,... so consecutive
            # mm1 stationaries target opposite PE row halves (LDWEIGHTS can
            # be pulled ahead of the in-flight matmul).
            ot_sb = outpool.tile([65, QL], F32, tag="ot")
            if alt_halves:
                ks = [x for pair in zip(range(32), range(32, 64)) for x in pair]
            else:
                ks = list(range(KT_TILES))
            batches = [ks[i:i + kb] for i in range(0, KT_TILES, kb)]
            assert n_qh == 2
            o2a = acpool.tile([65, QB], F32, tag="o2")
            o2b = acpool.tile([65, QB], F32, tag="o2")
            o2s = [o2a, o2b]
            for b, kset in enumerate(batches):
                for qh in range(n_qh):
                    sc = scpool.tile([128, QB * kb], F32, tag="sc")
                    for j, kk in enumerate(kset):
                        half = 0 if kk < 32 else 64
                        col = (kk % 32) * 128
                        nc.tensor.matmul(
                            sc[:, j * QB:(j + 1) * QB],
                            kt[half:half + 64, col:col + 128],
                            qt[half:half + 64, qh * QB:(qh + 1) * QB],
                            start=True, stop=True,
                        )
                    ex = expool.tile([128, QB * kb], MOV, tag="ex")
                    nc.scalar.activation(
                        ex[:, :len(kset) * QB], sc[:, :len(kset) * QB],
                        EXP, scale=0.125,
                    )
                    for j, kk in enumerate(kset):
                        nc.tensor.matmul(
                            o2s[qh][:],
                            va[:, kk * (D + 1):(kk + 1) * (D + 1)],
                            ex[:, j * QB:(j + 1) * QB],
                            start=(b == 0 and j == 0),
                            stop=(b == len(batches) - 1 and j == len(kset) - 1),
                        )
            for qh in range(n_qh):
                nc.vector.tensor_copy(ot_sb[:, qh * QB:(qh + 1) * QB],
                                      o2s[qh][:])
            nc.sync.dma_start(OT[:], ot_sb[:])

        def emit_iter():
            if mode == "kb4":
                emit_iter_kb4()
                return
            if mode in ("ilv", "ilv2"):
                emit_iter_ilv(alt_halves=(mode == "ilv2"))
                return
            ot_sb = outpool.tile([65, QL], F32, tag="ot")
            for qh in range(n_qh):
                o2 = acpool.tile([65, QB], F32, tag="o2")
                batches = []
                k = 0
                while k < KT_TILES:
                    batches.append((k, min(kb, KT_TILES - k)))
                    k += kb

                def emit_mm2(ex, k0, bsz):
                    for j in range(bsz):
                        kk = k0 + j
                        nc.tensor.matmul(
                            o2[:],
                            va[:, kk * (D + 1):(kk + 1) * (D + 1)],
                            ex[:, j * QB:(j + 1) * QB],
                            start=(kk == 0), stop=(kk == KT_TILES - 1),
                        )

                for k0, bsz in batches:
                    sc = scpool.tile([128, QB * kb], F32, tag="sc")
                    for j in range(bsz):
                        kk = k0 + j
                        half = 0 if kk < 32 else 64
                        col = (kk % 32) * 128
                        nc.tensor.matmul(
                            sc[:, j * QB:(j + 1) * QB],
                            kt[half:half + 64, col:col + 128],
                            qt[half:half + 64, qh * QB:(qh + 1) * QB],
                            start=True, stop=True,
                        )
                    if mode == "noexp":
                        continue
                    ex = expool.tile([128, QB * kb], MOV, tag="ex")
                    nc.scalar.activation(
                        ex[:, :bsz * QB], sc[:, :bsz * QB], EXP, scale=0.125,
                    )
                    if mode == "nomm2":
                        continue
                    emit_mm2(ex, k0, bsz)
                if mode == "full":
                    nc.vector.tensor_copy(ot_sb[:, qh * QB:(qh + 1) * QB], o2[:])
            if mode == "full":
                nc.sync.dma_start(OT[:], ot_sb[:])

        if loop_iters > 1:
            with tc.For_i(0, loop_iters, 1):
                for _ in range(iters):
                    emit_iter()
        else:
            for _ in range(iters):
                emit_iter()


def _build(iters=1, loop_iters=1, mov_dt="float16", num_devices=N_CORES,
           mode="ilv2", kb=KB, sc_bufs=2, ac_bufs=2, ex_bufs=4):
    key = ("nc", iters, loop_iters, mov_dt, num_devices, mode,
           kb, sc_bufs, ac_bufs, ex_bufs)
    if key in _CACHE:
        return _CACHE[key]
    import concourse.tile as tile
    from concourse import bacc, mybir

    F32 = mybir.dt.float32
    F32R = mybir.dt.float32r
    MOV = getattr(mybir.dt, mov_dt)
    nc = bacc.Bacc("TRN2", target_bir_lowering=False, debug=False,
                   num_devices=num_devices)
    QT = nc.dram_tensor("QT", [128, QL], MOV, kind="ExternalInput").ap()
    KT = nc.dram_tensor("KT", [128, N // 2], MOV, kind="ExternalInput").ap()
    VA = nc.dram_tensor("VA", [128, KT_TILES * (D + 1)], MOV,
                        kind="ExternalInput").ap()
    OT = nc.dram_tensor("OT", [65, QL], F32, kind="ExternalOutput").ap()
    build_body(nc, tile, mybir, QT, KT, VA, OT, iters=iters,
               loop_iters=loop_iters, mov_dt=mov_dt, mode=mode,
               kb=kb, sc_bufs=sc_bufs, ac_bufs=ac_bufs, ex_bufs=ex_bufs)
    nc.compile()
    _CACHE[key] = nc
    return nc


def prep_inputs(Q, K, V, mov_np=np.float16):
    """Host-side shard/pack. Returns per-core input maps."""
    Q = np.ascontiguousarray(np.asarray(Q, dtype=np.float32))
    K = np.ascontiguousarray(np.asarray(K, dtype=np.float32))
    V = np.ascontiguousarray(np.asarray(V, dtype=np.float32))

    KTf = K.T                                   # [64, 8192]
    KTp = np.concatenate([KTf[:, :N // 2], KTf[:, N // 2:]], axis=0)  # [128, 4096]
    KTp = np.ascontiguousarray(KTp.astype(mov_np))

    Vaug = np.ones((N, D + 1), dtype=np.float32)
    Vaug[:, :D] = V
    # [128, kt*(D+1)] : partition p, tile t -> V row t*128+p
    VAp = np.ascontiguousarray(
        Vaug.reshape(KT_TILES, 128, D + 1).transpose(1, 0, 2)
            .reshape(128, -1).astype(mov_np))

    in_maps = []
    for c in range(N_CORES):
        QTc = Q[c * QL:(c + 1) * QL].T          # [64, QL]
        QTd = np.ascontiguousarray(
            np.concatenate([QTc, QTc], axis=0).astype(mov_np))
        in_maps.append({"QT": QTd, "KT": KTp, "VA": VAp})
    return in_maps


def postprocess(results):
    """Divide by softmax denominator and transpose back, per core."""
    outs = []
    for c in range(N_CORES):
        OTc = results[c]["OT"]                  # [65, QL]
        outs.append((OTc[:D] / OTc[D:D + 1]).T)
    return np.ascontiguousarray(np.concatenate(outs, axis=0), dtype=np.float32)


def kernel(Q, K, V):
    import os
    # the NTFF trace path needs antenv.axon_hooks, absent on this client
    os.environ["BASS_NEVER_TRACE"] = "1"
    from concourse.bass_utils import run_bass_kernel_spmd

    nc = _build(iters=1)
    in_maps = prep_inputs(Q, K, V)
    res = run_bass_kernel_spmd(nc, in_maps, list(range(N_CORES)))
    return postprocess(res.results)


# revision 45
# speedup vs baseline: 419.3938x; 1.2140x over previous
"""Single-head attention  softmax(Q K^T / sqrt(64)) V  on 8 TRN2 NeuronCores.

Shapes: Q, K, V = [8192, 64] f32; output [8192, 64] f32.

Sharding: Q rows split 8 ways (1024 rows/core); K, V replicated.

Per-core algorithm (transpose-free layout):
  - scoresT[k, q] tiles ([128 k x 512 q]) via matmul with lhsT = K^T tile
    (d=64 contraction on partitions), rhs = Q^T block.  fp16 operands:
    full-rate PE streaming (fp32 is 4x slower, fp32r measured 2x slower on
    HW despite the cost model's claim) at ~1e-3 overall rel err.
  - exp on ScalarE straight out of PSUM, with the 1/sqrt(64) scale fused
    into the activation's free affine.  No max-subtraction: scores are
    ~N(0,1) so exp is safely in fp32 range, matching softmax exactly.
  - second matmul accumulates [V | 1] (ones-augmented, k-tiled) against the
    exp tiles into a [65, 512] PSUM accumulator: rows 0..63 = unnormalized
    out^T, row 64 = softmax denominator.  One accumulation over all 64
    k-tiles per q-block.
  - device returns OT [65, 1024] per core; host divides by row 64 and
    transposes.  Host also pre-transposes/packs Q^T, K^T, [V|1] so the
    device does zero transposes.

K^T is packed [128, 4096] (two 64-row halves side by side) and Q^T is
duplicated into both partition halves so k-tiles >= 32 contract on array
rows 64..127 (full DMA width, base-partition rule satisfied).

Default schedule ("ilv2"): the two 512-wide q-block chains are interleaved
batch-by-batch (independent work fills cross-engine sem-wait gaps), and
k-tiles are enumerated 0,32,1,33,... so consecutive mm1 stationaries target
opposite PE row halves, letting the PE's reorder window pull each LDWEIGHTS
ahead of the in-flight matmul (~12 us/core on HW vs sequential halves).
"""

import numpy as np

N = 8192
D = 64
N_CORES = 8
QL = N // N_CORES          # 1024 q rows per core
KT_TILES = N // 128        # 64 k-tiles of 128
QB = 512                   # q block (one PSUM bank of f32)
KB = 3                     # k-tiles per exp batch (3 PSUM banks)

_CACHE = {}


def build_body(nc, tile, mybir, QT, KT, VA, OT, iters=1, loop_iters=1,
               mov_dt="float16", mode="ilv2", kb=KB, sc_bufs=2, ac_bufs=2,
               ex_bufs=4):
    """Emit the per-core program. QT [128, QL] (Q^T/dup), KT [128, N/2]
    (packed K^T), VA [128, 64*65] (k-tiled [V|1]), OT [65, QL].

    iters: unrolled repetitions of the whole computation (timing runs).
    loop_iters: hardware For_i loop around those repetitions (timing runs).
    """
    from contextlib import ExitStack

    F32 = mybir.dt.float32
    F32R = mybir.dt.float32r
    MOV = getattr(mybir.dt, mov_dt)
    EXP = mybir.ActivationFunctionType.Exp
    n_qh = QL // QB

    with tile.TileContext(nc) as tc, ExitStack() as ctx:
        sing = ctx.enter_context(tc.tile_pool(name="sing", bufs=1))
        scpool = ctx.enter_context(
            tc.tile_pool(name="sc", bufs=sc_bufs, space="PSUM"))
        acpool = ctx.enter_context(
            tc.tile_pool(name="ac", bufs=ac_bufs, space="PSUM"))
        expool = ctx.enter_context(tc.tile_pool(name="ex", bufs=ex_bufs))
        outpool = ctx.enter_context(tc.tile_pool(name="ot", bufs=2))

        qt = sing.tile([128, QL], MOV, tag="qt")
        kt = sing.tile([128, N // 2], MOV, tag="kt")
        va = sing.tile([128, KT_TILES * (D + 1)], MOV, tag="va")

        nc.sync.dma_start(qt[:], QT[:])
        for i in range(8):
            w = (N // 2) // 8
            nc.sync.dma_start(kt[:, i * w:(i + 1) * w], KT[:, i * w:(i + 1) * w])
        for i in range(8):
            w = KT_TILES * (D + 1) // 8
            nc.sync.dma_start(va[:, i * w:(i + 1) * w], VA[:, i * w:(i + 1) * w])

        def emit_iter_kb4():
            # scores [128, 2048] x2 slots = 8 banks; mm2 batch partials are
            # written back into bank 0 of the spent scores slot, then DVE
            # accumulates them into an SBUF accumulator.
            ot_sb = outpool.tile([65, QL], F32, tag="ot")
            for qh in range(n_qh):
                acc = expool.tile([65, QB], F32, tag="acc")
                nc.vector.memset(acc[:], 0.0)
                for b in range(KT_TILES // 4):
                    k0 = b * 4
                    sc = scpool.tile([128, QB * 4], F32, tag="sc")
                    for j in range(4):
                        kk = k0 + j
                        half = 0 if kk < 32 else 64
                        col = (kk % 32) * 128
                        nc.tensor.matmul(
                            sc[:, j * QB:(j + 1) * QB],
                            kt[half:half + 64, col:col + 128],
                            qt[half:half + 64, qh * QB:(qh + 1) * QB],
                            start=True, stop=True,
                        )
                    ex = expool.tile([128, QB * 4], MOV, tag="ex")
                    nc.scalar.activation(ex[:], sc[:], EXP, scale=0.125)
                    part = sc[0:65, 0:QB]
                    for j in range(4):
                        kk = k0 + j
                        nc.tensor.matmul(
                            part,
                            va[:, kk * (D + 1):(kk + 1) * (D + 1)],
                            ex[:, j * QB:(j + 1) * QB],
                            start=(j == 0), stop=(j == 3),
                        )
                    nc.vector.tensor_add(acc[:], acc[:], part)
                nc.vector.tensor_copy(ot_sb[:, qh * QB:(qh + 1) * QB], acc[:])
            nc.sync.dma_start(OT[:], ot_sb[:])

        def emit_iter_ilv(alt_halves):
            # Interleave the two independent q-block chains batch-by-batch so
            # each engine has the other chain's work to fill sem-wait gaps.
            # alt_halves: enumerate k-tiles as 0,32,1,33,... so consecutive
            # mm1 stationaries target opposite PE row halves (LDWEIGHTS can
            # be pulled ahead of the in-flight matmul).
            ot_sb = outpool.tile([65, QL], F32, tag="ot")
            if alt_halves:
                ks = [x for pair in zip(range(32), range(32, 64)) for x in pair]
            else:
                ks = list(range(KT_TILES))
            batches = [ks[i:i + kb] for i in range(0, KT_TILES, kb)]
            assert n_qh == 2
            o2a = acpool.tile([65, QB], F32, tag="o2")
            o2b = acpool.tile([65, QB], F32, tag="o2")
            o2s = [o2a, o2b]
            for b, kset in enumerate(batches):
                for qh in range(n_qh):
                    sc = scpool.tile([128, QB * kb], F32, tag="sc")
                    for j, kk in enumerate(kset):
                        half = 0 if kk < 32 else 64
                        col = (kk % 32) * 128
                        nc.tensor.matmul(
                            sc[:, j * QB:(j + 1) * QB],
                            kt[half:half + 64, col:col + 128],
                            qt[half:half + 64, qh * QB:(qh + 1) * QB],
                            start=True, stop=True,
                        )
                    ex = expool.tile([128, QB * kb], MOV, tag="ex")
                    nc.scalar.activation(
                        ex[:, :len(kset) * QB], sc[:, :len(kset) * QB],
                        EXP, scale=0.125,
                    )
                    for j, kk in enumerate(kset):
                        nc.tensor.matmul(
                            o2s[qh][:],
                            va[:, kk * (D + 1):(kk + 1) * (D + 1)],
                            ex[:, j * QB:(j + 1) * QB],
                            start=(b == 0 and j == 0),
                            stop=(b == len(batches) - 1 and j == len(kset) - 1),
                        )
            for qh in range(n_qh):
                nc.vector.tensor_copy(ot_sb[:, qh * QB:(qh + 1) * QB],
                                      o2s[qh][:])
            nc.sync.dma_start(OT[:], ot_sb[:])

        def emit_iter_ilv3():
            # KB=4: scores [128, 2048] x2 slots = all 8 banks.  The mm2
            # accumulator moves to SBUF: each batch's partial [65, 512] is
            # accumulated in PSUM bank 0 of the spent scores slot
            # (start/stop per batch), then DVE adds it into the ot_sb slice.
            # Chains interleaved + alternating row halves as in ilv2.
            ot_sb = outpool.tile([65, QL], F32, tag="ot")
            ks = [x for pair in zip(range(32), range(32, 64)) for x in pair]
            batches = [ks[i:i + 4] for i in range(0, KT_TILES, 4)]
            for qh in range(n_qh):
                nc.vector.memset(ot_sb[:, qh * QB:(qh + 1) * QB], 0.0)
            for b, kset in enumerate(batches):
                for qh in range(n_qh):
                    sc = scpool.tile([128, QB * 4], F32, tag="sc")
                    for j, kk in enumerate(kset):
                        half = 0 if kk < 32 else 64
                        col = (kk % 32) * 128
                        nc.tensor.matmul(
                            sc[:, j * QB:(j + 1) * QB],
                            kt[half:half + 64, col:col + 128],
                            qt[half:half + 64, qh * QB:(qh + 1) * QB],
                            start=True, stop=True,
                        )
                    ex = expool.tile([128, QB * 4], MOV, tag="ex")
                    nc.scalar.activation(ex[:], sc[:], EXP, scale=0.125)
                    part = sc[0:65, 0:QB]
                    for j, kk in enumerate(kset):
                        nc.tensor.matmul(
                            part,
                            va[:, kk * (D + 1):(kk + 1) * (D + 1)],
                            ex[:, j * QB:(j + 1) * QB],
                            start=(j == 0), stop=(j == 3),
                        )
                    osl = ot_sb[:, qh * QB:(qh + 1) * QB]
                    nc.vector.tensor_add(osl, osl, part)
            nc.sync.dma_start(OT[:], ot_sb[:])

        def emit_iter():
            if mode == "kb4":
                emit_iter_kb4()
                return
            if mode == "ilv3":
                emit_iter_ilv3()
                return
            if mode in ("ilv", "ilv2"):
                emit_iter_ilv(alt_halves=(mode == "ilv2"))
                return
            ot_sb = outpool.tile([65, QL], F32, tag="ot")
            for qh in range(n_qh):
                o2 = acpool.tile([65, QB], F32, tag="o2")
                batches = []
                k = 0
                while k < KT_TILES:
                    batches.append((k, min(kb, KT_TILES - k)))
                    k += kb

                def emit_mm2(ex, k0, bsz):
                    for j in range(bsz):
                        kk = k0 + j
                        nc.tensor.matmul(
                            o2[:],
                            va[:, kk * (D + 1):(kk + 1) * (D + 1)],
                            ex[:, j * QB:(j + 1) * QB],
                            start=(kk == 0), stop=(kk == KT_TILES - 1),
                        )

                for k0, bsz in batches:
                    sc = scpool.tile([128, QB * kb], F32, tag="sc")
                    for j in range(bsz):
                        kk = k0 + j
                        half = 0 if kk < 32 else 64
                        col = (kk % 32) * 128
                        nc.tensor.matmul(
                            sc[:, j * QB:(j + 1) * QB],
                            kt[half:half + 64, col:col + 128],
                            qt[half:half + 64, qh * QB:(qh + 1) * QB],
                            start=True, stop=True,
                        )
                    if mode == "noexp":
                        continue
                    ex = expool.tile([128, QB * kb], MOV, tag="ex")
                    nc.scalar.activation(
                        ex[:, :bsz * QB], sc[:, :bsz * QB], EXP, scale=0.125,
                    )
                    if mode == "nomm2":
                        continue
                    emit_mm2(ex, k0, bsz)
                if mode == "full":
                    nc.vector.tensor_copy(ot_sb[:, qh * QB:(qh + 1) * QB], o2[:])
            if mode == "full":
                nc.sync.dma_start(OT[:], ot_sb[:])

        if loop_iters > 1:
            with tc.For_i(0, loop_iters, 1):
                for _ in range(iters):
                    emit_iter()
        else:
            for _ in range(iters):
                emit_iter()


def _build(iters=1, loop_iters=1, mov_dt="float16", num_devices=N_CORES,
           mode="ilv2", kb=KB, sc_bufs=2, ac_bufs=2, ex_bufs=4):
    key = ("nc", iters, loop_iters, mov_dt, num_devices, mode,
           kb, sc_bufs, ac_bufs, ex_bufs)
    if key in _CACHE:
        return _CACHE[key]
    import concourse.tile as tile
    from concourse import bacc, mybir

    F32 = mybir.dt.float32
    F32R = mybir.dt.float32r
    MOV = getattr(mybir.dt, mov_dt)
    nc = bacc.Bacc("TRN2", target_bir_lowering=False, debug=False,
                   num_devices=num_devices)
    QT = nc.dram_tensor("QT", [128, QL], MOV, kind="ExternalInput").ap()
    KT = nc.dram_tensor("KT", [128, N // 2], MOV, kind="ExternalInput").ap()
    VA = nc.dram_tensor("VA", [128, KT_TILES * (D + 1)], MOV,
                        kind="ExternalInput").ap()
    OT = nc.dram_tensor("OT", [65, QL], F32, kind="ExternalOutput").ap()
    build_body(nc, tile, mybir, QT, KT, VA, OT, iters=iters,
               loop_iters=loop_iters, mov_dt=mov_dt, mode=mode,
               kb=kb, sc_bufs=sc_bufs, ac_bufs=ac_bufs, ex_bufs=ex_bufs)
    nc.compile()
    _CACHE[key] = nc
    return nc


def prep_inputs(Q, K, V, mov_np=np.float16):
    """Host-side shard/pack. Returns per-core input maps."""
    Q = np.ascontiguousarray(np.asarray(Q, dtype=np.float32))
    K = np.ascontiguousarray(np.asarray(K, dtype=np.float32))
    V = np.ascontiguousarray(np.asarray(V, dtype=np.float32))

    KTf = K.T                                   # [64, 8192]
    KTp = np.concatenate([KTf[:, :N // 2], KTf[:, N // 2:]], axis=0)  # [128, 4096]
    KTp = np.ascontiguousarray(KTp.astype(mov_np))

    Vaug = np.ones((N, D + 1), dtype=np.float32)
    Vaug[:, :D] = V
    # [128, kt*(D+1)] : partition p, tile t -> V row t*128+p
    VAp = np.ascontiguousarray(
        Vaug.reshape(KT_TILES, 128, D + 1).transpose(1, 0, 2)
            .reshape(128, -1).astype(mov_np))

    in_maps = []
    for c in range(N_CORES):
        QTc = Q[c * QL:(c + 1) * QL].T          # [64, QL]
        QTd = np.ascontiguousarray(
            np.concatenate([QTc, QTc], axis=0).astype(mov_np))
        in_maps.append({"QT": QTd, "KT": KTp, "VA": VAp})
    return in_maps


def postprocess(results):
    """Divide by softmax denominator and transpose back, per core."""
    outs = []
    for c in range(N_CORES):
        OTc = results[c]["OT"]                  # [65, QL]
        outs.append((OTc[:D] / OTc[D:D + 1]).T)
    return np.ascontiguousarray(np.concatenate(outs, axis=0), dtype=np.float32)


def kernel(Q, K, V):
    import os
    # the NTFF trace path needs antenv.axon_hooks, absent on this client
    os.environ["BASS_NEVER_TRACE"] = "1"
    from concourse.bass_utils import run_bass_kernel_spmd

    nc = _build(iters=1)
    in_maps = prep_inputs(Q, K, V)
    res = run_bass_kernel_spmd(nc, in_maps, list(range(N_CORES)))
    return postprocess(res.results)
